# revision 31
# baseline (speedup 1.0000x reference)
"""Causal self-attention (B=16, T=1024, C=768, H=12) on 8 NeuronCores.

Strategy: data-parallel over batch (2 batches per core, no collectives).

v2 redesign for PE density (HAM clock gate wants zero PE-idle gaps):
  - All matmuls in bf16 (fp32 PSUM accumulate): 1 cycle/row at any width,
    FWL weight loads, half SBUF/DMA traffic.
  - Per head: S^T tiles stream through 2 PSUM slots with a 2-tile lookahead
    so the PE never waits on ScalarE's exp.
  - Next pair's QKV projection groups are interleaved INTO the attention
    loop as PE filler (ScalarE exp is the slow engine in phase D; the
    filler keeps the PE warm while exp catches up).
  - Softmax normalization is deferred to a batch-end tail: denominator rows
    are DMA-gathered into l_all[12, T], one reciprocal_approx_fast serves
    the whole batch, and a selector-matrix matmul (eye12 (x) ones64)
    broadcasts each head's reciprocal row to 64 partitions. This kills the
    24 single-lane DVE RECIPROCALs (6.5us each!) of v1.
  - Bias adds moved off ScalarE (DVE tensor_scalar with per-partition AP)
    so ScalarE does exp only.
"""

import os
import numpy as np
from collections import deque
from contextlib import ExitStack

import concourse.bass as bass
import concourse.mybir as mybir
import concourse.tile as tile
from concourse.bass import ds, ts
from concourse.bass_utils import run_bass_kernel_spmd

F32 = mybir.dt.float32
F32R = mybir.dt.float32r
BF = mybir.dt.bfloat16

B, T, C, H = 16, 1024, 768, 12
D = C // H           # 64
NCORES = 8
B_LOC = B // NCORES  # 2
KT = C // 128        # 6 contraction tiles
TT = T // 128        # 8 token tiles
NPAIR = H // 2       # 6 head pairs
EXP = mybir.ActivationFunctionType.Exp


def split_multi_waits(nc):
    """Hoist surplus sync waits onto standalone EventSemaphore instructions.

    The walrus build in this environment rejects any instruction carrying
    more than one sync wait ("Too many sync wait commands"). Engine queues
    execute in order, so waiting on each semaphore in a preceding
    EventSemaphore instruction is equivalent to waiting on all of them at
    the original instruction.
    """
    n_split = 0
    for f in nc.m.functions:
        for blk in f.blocks:
            out = []
            for inst in blk.instructions:
                si = inst.sync_info
                if si is not None and si.on_wait and len(si.on_wait) > 1:
                    waits = list(si.on_wait)
                    for w in waits[:-1]:
                        n_split += 1
                        ev = mybir.InstEventSemaphore(
                            name=f"I-waitsplit-{n_split}",
                            ins=[],
                            outs=[],
                            engine=inst.engine,
                            sync_info=mybir.SyncInfo(on_wait=[w], on_update=[]),
                        )
                        out.append(ev)
                    si.on_wait = waits[-1:]
                out.append(inst)
            blk.instructions[:] = out
    return n_split


def build_program(split_waits=True, level=None):
    """split_waits: apply the multi-wait splitting (required for neuronx-cc
    codegen, but the CoreSim race detector rejects the synthetic
    EventSemaphore instructions — pass False when simulating)."""
    if level is None:
        level = int(os.environ.get("BUILD_LEVEL", "5"))
    nc = bass.Bass()
    x = nc.declare_dram_parameter("x", [B_LOC, T, C], BF, isOutput=False)
    wqkv = nc.declare_dram_parameter("wqkv", [C, 3 * C], BF, isOutput=False)
    wproj = nc.declare_dram_parameter("wproj", [C, C], BF, isOutput=False)
    bqkt = nc.declare_dram_parameter("bqkt", [128, 2 * NPAIR], F32, isOutput=False)
    bvbc = nc.declare_dram_parameter("bvbc", [128, C], F32, isOutput=False)
    bobc = nc.declare_dram_parameter("bobc", [128, C], F32, isOutput=False)
    maskb = nc.declare_dram_parameter("maskb", [128, 128], BF, isOutput=False)
    identb = nc.declare_dram_parameter("identb", [128, 128], BF, isOutput=False)
    sel12 = nc.declare_dram_parameter("sel12", [128, C], F32, isOutput=False)
    out = nc.declare_dram_parameter("out", [B_LOC, T, C], F32, isOutput=True)

    with tile.TileContext(nc) as tc, ExitStack() as ctx, \
            nc.allow_low_precision(reason="bf16 matmul pipeline"):
        consts = ctx.enter_context(tc.tile_pool(name="consts", bufs=1))
        wq_pool = ctx.enter_context(tc.tile_pool(name="wq", bufs=1))
        wp_pool = ctx.enter_context(tc.tile_pool(name="wp", bufs=1))
        xt_pool = ctx.enter_context(tc.tile_pool(name="xt", bufs=1))
        qk_pool = ctx.enter_context(tc.tile_pool(name="qk", bufs=3))
        va_pool = ctx.enter_context(tc.tile_pool(name="va", bufs=1))
        pexp = ctx.enter_context(tc.tile_pool(name="pexp", bufs=4))
        lpool = ctx.enter_context(tc.tile_pool(name="lpool", bufs=1))
        yraw_pool = ctx.enter_context(tc.tile_pool(name="yraw", bufs=1))
        ystg_pool = ctx.enter_context(tc.tile_pool(name="ystg", bufs=2))
        yt_pool = ctx.enter_context(tc.tile_pool(name="yt", bufs=1))
        ostage = ctx.enter_context(tc.tile_pool(name="ostage", bufs=3))
        # PSUM: pco 2x1 bank + st 2x2 banks + ypool 1x2 banks = 8 banks
        pco = ctx.enter_context(tc.tile_pool(name="pco", bufs=2, space="PSUM"))
        st_pool = ctx.enter_context(tc.tile_pool(name="st", bufs=2, space="PSUM"))
        ypool = ctx.enter_context(tc.tile_pool(name="ypool", bufs=1, space="PSUM"))

        # ---- constants -------------------------------------------------
        ident_sb = consts.tile([128, 128], BF)
        nc.sync.dma_start(ident_sb[:], identb[:])
        mask_sb = consts.tile([128, 128], BF)
        nc.sync.dma_start(mask_sb[:], maskb[:])
        xstage = ctx.enter_context(tc.tile_pool(name="xstage", bufs=2))
        bqk_sb = consts.tile([128, 2 * NPAIR], F32)
        bvbc_sb = consts.tile([128, C], F32)
        bobc_sb = consts.tile([128, C], F32)
        sel_sb = consts.tile([128, C], F32R)

        # ---- weights: bf16 from host; one strided DMA per tensor -------
        # (DMA issue on the sync queue costs ~0.65us each, so batch them;
        # emitted AFTER the batch-0 x DMA so phase A is never starved.)
        wqall = wq_pool.tile([128, KT, 3 * C], BF, name="wqall")
        wpall = wp_pool.tile([128, KT, C], BF, name="wpall")
        wq = [wqall[:, k, :] for k in range(KT)]
        wp = [wpall[:, k, :] for k in range(KT)]

        def emit_weight_dmas():
            wqkv3 = wqkv.rearrange("(k p) c -> p k c", p=128)
            nc.sync.dma_start(wqall[:, :, 2 * C :], wqkv3[:, :, 2 * C :])
            nc.sync.dma_start(wqall[:, :, 0 : 2 * C], wqkv3[:, :, 0 : 2 * C])
            nc.sync.dma_start(bqk_sb[:], bqkt[:])
            nc.sync.dma_start(bvbc_sb[:], bvbc[:])
            nc.sync.dma_start(wpall[:], wproj.rearrange("(k p) c -> p k c", p=128))
            nc.sync.dma_start(bobc_sb[:], bobc[:])
            nc.sync.dma_start(sel_sb[:], sel12[:].bitcast(F32R))

        xT = [None] * KT
        va_tiles = [None] * TT
        yraw = {}
        yTt = [None] * NPAIR
        cur_qk = {}
        l_all = None
        l5 = None
        rec = None
        rec5 = None

        # ---- emit helpers ---------------------------------------------
        xfull = [None]

        def emit_Adma(b):
            xs = xstage.tile([128, TT, C], BF, tag="xs", name="xs")
            nc.sync.dma_start(xs[:], x[b].rearrange("(t p) c -> p t c", p=128))
            xfull[0] = xs

        def emit_Astep(b, tt):
            xs = xfull[0]
            for k in range(KT):
                ptr = pco.tile([128, 512], F32, tag="mm", name="ptr")
                pbf = ptr.bitcast(BF)
                nc.tensor.transpose(pbf[:, 0:128], xs[:, tt, ts(k, 128)], ident_sb[:])
                nc.vector.tensor_copy(xT[k][:, ts(tt, 128)], pbf[:, 0:128])

        def emit_Cgroup(b, tt, half):
            pv = pco.tile([128, 384], F32, tag="mm", name="pv")
            for k in range(KT):
                nc.tensor.matmul(
                    pv[:],
                    lhsT=xT[k][:, ts(tt, 128)],
                    rhs=wq[k][:, ds(2 * C + 384 * half, 384)],
                    start=(k == 0),
                    stop=(k == KT - 1),
                )
            va3 = va_tiles[tt].rearrange("p (h e) -> p h e", e=D + 1)
            nc.vector.tensor_add(
                va3[:, ds(6 * half, 6), 0:D],
                pv[:].rearrange("p (h e) -> p h e", e=D),
                bvbc_sb[:, ds(384 * half, 384)].rearrange("p (h e) -> p h e", e=D),
            )

        def emit_Bgroup(b, p, which, half, xts=None):
            if xts is None:
                xts = xT
            if p not in cur_qk:
                qT = qk_pool.tile([128, T], BF, tag="qT", name="qT")
                kTt = qk_pool.tile([128, T], BF, tag="kT", name="kTt")
                cur_qk[p] = (qT, kTt)
            dst = cur_qk[p][which]
            pq = pco.tile([128, 512], F32, tag="mm", name="pq")
            colbase = 128 * p + which * C
            for k in range(KT):
                nc.tensor.matmul(
                    pq[:],
                    lhsT=wq[k][:, ds(colbase, 128)],
                    rhs=xts[k][:, ds(512 * half, 512)],
                    start=(k == 0),
                    stop=(k == KT - 1),
                )
            j = p + which * NPAIR
            nc.vector.tensor_scalar_add(
                dst[:, ds(512 * half, 512)], pq[:], bqk_sb[:, ds(j, 1)]
            )

        pending = deque()

        def pop_filler(n=1):
            for _ in range(n):
                if pending:
                    pending.popleft()()

        def emit_D_head(b, p, h2):
            h = 2 * p + h2
            pb = D * h2
            qT, kTt = cur_qk[p]
            py = ypool.tile([D + 1, T], F32, tag="y", name="py")
            sts = {}

            def est(i):
                cstart = 128 * i
                wtot = T - cstart
                st = st_pool.tile([128, wtot], F32, tag="st", name="st")
                lc = 0
                while lc < wtot:
                    w = min(512, wtot - lc)
                    nc.tensor.matmul(
                        st[:, ds(lc, w)],
                        lhsT=kTt[ds(pb, D), ts(i, 128)],
                        rhs=qT[ds(pb, D), ds(cstart + lc, w)],
                        start=True,
                        stop=True,
                    )
                    lc += w
                sts[i] = st

            est(0)
            est(1)
            for i in range(TT):
                cstart = 128 * i
                wtot = T - cstart
                pe = pexp.tile([128, wtot], BF, tag="pe", name="pe")
                nc.scalar.activation(pe[:], sts[i][:], EXP, scale=0.125)
                nc.gpsimd.tensor_mul(pe[:, 0:128], pe[:, 0:128], mask_sb[:])
                if i + 2 < TT:
                    est(i + 2)
                if i % 2 == 1:
                    pop_filler()
                cs = cstart
                while cs < T:
                    w = min(512 - (cs % 512), T - cs)
                    nc.tensor.matmul(
                        py[:, ds(cs, w)],
                        lhsT=va_tiles[i][:, ds((D + 1) * h, D + 1)],
                        rhs=pe[:, ds(cs - cstart, w)],
                        start=(i == 0),
                        stop=(i == TT - 1),
                        skip_group_check=True,
                    )
                    cs += w
            # l row: PSUM part-64 -> SBUF part-64 (same-base DVE copy), then
            # SBUF->SBUF DMA does the cross-partition move into l_all[h].
            # Pair 5 lands in its own base-0 tile so its reciprocal can run
            # after the early rows-0:9 reciprocal (start partition must be 0).
            lstg = ystg_pool.tile([D + 1, T], F32, tag="lstg", name="lstg")
            nc.vector.tensor_copy(lstg[ds(D, 1), :], py[ds(D, 1), :])
            if h >= 10:
                nc.sync.dma_start(l5[ds(h - 10, 1), :], lstg[ds(D, 1), :])
            else:
                nc.sync.dma_start(l_all[ds(h, 1), :], lstg[ds(D, 1), :])
            yr = yraw_pool.tile([D, T], BF, tag=f"yr{h}", name=f"yr{h}")
            nc.vector.tensor_copy(yr[:], py[ds(0, D), :])
            yraw[h] = yr

        cur_ystg = {}

        def emit_norm_pair(b, p, half):
            for h2 in range(2):
                h = 2 * p + h2
                if h2 == 0:
                    if half == 0:
                        yt = yt_pool.tile([128, T], BF, tag=f"yT{p}", name=f"yT{p}")
                        yTt[p] = yt
                    tgt = yTt[p]
                else:
                    if half == 0:
                        cur_ystg[p] = ystg_pool.tile(
                            [D, T], BF, tag="ystg", name="ystg", bufs=6
                        )
                    tgt = cur_ystg[p]
                pbt = pco.tile([D, 512], F32, tag="mm", name="pbt")
                if p < NPAIR - 1:
                    sel_ap = sel_sb[ds(0, 10), ds(64 * h, 64)]
                    rec_ap = rec[ds(0, 10), ds(512 * half, 512)]
                else:
                    # sel rows 0-1 over cols 0:128 are eye2 (x) ones64
                    sel_ap = sel_sb[ds(0, 2), ds(64 * h2, 64)]
                    rec_ap = rec5[ds(0, 2), ds(512 * half, 512)]
                nc.tensor.matmul(
                    pbt[:], lhsT=sel_ap, rhs=rec_ap, start=True, stop=True
                )
                nc.vector.tensor_mul(
                    tgt[ds(0, D), ds(512 * half, 512)],
                    yraw[h][:, ds(512 * half, 512)],
                    pbt[:],
                )
                if h2 == 1:
                    nc.sync.dma_start(
                        yTt[p][ds(D, D), ds(512 * half, 512)],
                        tgt[ds(0, D), ds(512 * half, 512)],
                    )

        def emit_Egroup(b, tt, half, yt_fin):
            po = pco.tile([128, 384], F32, tag="mm", name="po")
            for k in range(KT):
                nc.tensor.matmul(
                    po[:],
                    lhsT=yt_fin[k][:, ts(tt, 128)],
                    rhs=wp[k][:, ds(384 * half, 384)],
                    start=(k == 0),
                    stop=(k == KT - 1),
                )
            ot = ostage.tile([128, 384], F32, tag="ot", name="ot")
            nc.vector.tensor_add(ot[:], po[:], bobc_sb[:, ds(384 * half, 384)])
            nc.sync.dma_start(out[b, ts(tt, 128), ds(384 * half, 384)], ot[:])

        def make_CB0(b):
            """Closures: va-tile setup, then C groups with B(pair 0) groups
            interleaved every 4th slot. Resets cur_qk when the first runs."""

            def setup():
                for tt in range(TT):
                    va = va_pool.tile(
                        [128, H * (D + 1)], BF, tag=f"va{tt}", name=f"va{tt}"
                    )
                    va3 = va.rearrange("p (h e) -> p h e", e=D + 1)
                    nc.vector.memset(va3[:, :, D : D + 1], 1.0)
                    va_tiles[tt] = va

            items = [setup]
            for tt in range(TT):
                for half in range(2):
                    items.append(lambda tt=tt, half=half: emit_Cgroup(b, tt, half))
            return items

        def emit_CB0(b):
            for f in make_CB0(b):
                f()

        # ---- program ---------------------------------------------------
        def emit_batch(b, skipA, skipCB=False):
            nonlocal l_all, l5, rec, rec5
            if not skipA:
                for k in range(KT):
                    xT[k] = xt_pool.tile([128, T], BF, tag=f"xT{k}", name=f"xT{k}")
                emit_Adma(b)
                if b == 0:
                    emit_weight_dmas()
                for tt in range(TT):
                    emit_Astep(b, tt)
            if level <= 1:
                for k in range(KT):
                    nc.sync.dma_start(out[b, ts(k, 128), 0:512], xT[k][:, :].bitcast(F32))
                return
            if not skipCB:
                cur_qk.clear()
                cb_items = make_CB0(b)
                for n, f in enumerate(cb_items):
                    f()
                    if n in (4, 8, 12, 16):
                        g = (n - 4) // 4
                        emit_Bgroup(b, 0, g // 2, g % 2)
            if level <= 2:
                for tt in range(TT):
                    nc.sync.dma_start(out[b, ts(tt, 128), 0:390], va_tiles[tt][:, :].bitcast(F32))
                return
            if level <= 3:
                for p in range(1, NPAIR):
                    for g in range(4):
                        emit_Bgroup(b, p, g // 2, g % 2)
                for p in range(NPAIR):
                    qT, kTt = cur_qk[p]
                    nc.sync.dma_start(out[b, ts(p, 128), 0:512], qT[:, :].bitcast(F32))
                    nc.sync.dma_start(out[b, ts(p, 128), 512:768], kTt[:, 0:512].bitcast(F32))
                return
            l_all = lpool.tile([10, T], F32, tag="l_all", name="l_all")
            l5 = lpool.tile([2, T], F32, tag="l5", name="l5")
            for p in range(NPAIR):
                if p < NPAIR - 1:
                    pnext = p + 1
                    xsnap = list(xT)
                    for g in range(4):
                        pending.append(
                            lambda g=g, pnext=pnext, xs=xsnap: emit_Bgroup(
                                b, pnext, g // 2, g % 2, xs
                            )
                        )
                    if p == NPAIR - 2 and b == 0 and level >= 5:
                        # stage half the next batch's transposes here so the
                        # pair-5 DVE queue stays short for the l/recip chain
                        for k in range(KT):
                            xT[k] = xt_pool.tile(
                                [128, T], BF, tag=f"xT{k}", name=f"xT{k}"
                            )
                        emit_Adma(1)
                        for tt in range(4):
                            pending.append(lambda tt=tt: emit_Astep(1, tt))
                elif b == 0 and level >= 5:
                    for tt in range(4, TT):
                        pending.append(lambda tt=tt: emit_Astep(1, tt))
                    cur_qk.pop(0, None)
                    xsnap = list(xT)
                    for g in range(4):
                        pending.append(
                            lambda g=g, xs=xsnap: emit_Bgroup(
                                1, 0, g // 2, g % 2, xs
                            )
                        )
                if p == NPAIR - 1:
                    # pairs 0-4 denominators are in: reciprocal them during
                    # the last pair's attention so only pair 5's rows remain
                    rec = lpool.tile([10, T], F32R, tag="rec", name="rec")
                    rec5 = lpool.tile([2, T], F32R, tag="rec5", name="rec5")
                    nc.vector.reciprocal(rec[ds(0, 10), 0:512], l_all[ds(0, 10), 0:512])
                    nc.vector.reciprocal(rec[ds(0, 10), 512:T], l_all[ds(0, 10), 512:T])
                emit_D_head(b, p, 0)
                emit_D_head(b, p, 1)
            pop_filler(len(pending))
            if b == 0:
                keep = cur_qk.get(0)
                cur_qk.clear()
                if keep is not None:
                    cur_qk[0] = keep
            nc.vector.reciprocal(rec5[ds(0, 2), 0:512], l5[ds(0, 2), 0:512])
            nc.vector.reciprocal(rec5[ds(0, 2), 512:T], l5[ds(0, 2), 512:T])
            for p in range(NPAIR):
                emit_norm_pair(b, p, 0)
            if b == 0:
                emit_CB0(1)
            if level <= 4:
                for p in range(NPAIR):
                    emit_norm_pair(b, p, 1)
                for p in range(NPAIR):
                    nc.sync.dma_start(out[b, ts(p, 128), 0:512], yTt[p][:, :].bitcast(F32))
                return
            yt_fin = list(yTt)
            # E for tt 0-3 only needs the first 512 cols of yTt: interleave
            # with the second normalization half so the PE never waits.
            for tt in range(4):
                for half in range(2):
                    emit_Egroup(b, tt, half, yt_fin)
            for p in range(NPAIR):
                emit_norm_pair(b, p, 1)
            for tt in range(4, TT):
                for half in range(2):
                    emit_Egroup(b, tt, half, yt_fin)

        emit_batch(0, skipA=False)
        if level >= 5:
            emit_batch(1, skipA=True, skipCB=True)

    if split_waits:
        split_multi_waits(nc)
    return nc


def make_in_maps(x, W_qkv, b_qkv, W_proj, b_proj):
    x = np.ascontiguousarray(np.asarray(x, dtype=np.float32))
    W_qkv = np.ascontiguousarray(np.asarray(W_qkv, dtype=np.float32))
    b_qkv = np.asarray(b_qkv, dtype=np.float32)
    W_proj = np.ascontiguousarray(np.asarray(W_proj, dtype=np.float32))
    b_proj = np.asarray(b_proj, dtype=np.float32)

    import ml_dtypes

    x16 = x.astype(ml_dtypes.bfloat16)
    wqkv16 = np.ascontiguousarray(W_qkv.astype(ml_dtypes.bfloat16))
    wproj16 = np.ascontiguousarray(W_proj.astype(ml_dtypes.bfloat16))
    bqkt = np.ascontiguousarray(b_qkv[: 2 * C].reshape(2 * NPAIR, 128).T)
    bvbc = np.ascontiguousarray(np.tile(b_qkv[2 * C :].reshape(1, C), (128, 1)))
    bobc = np.ascontiguousarray(np.tile(b_proj.reshape(1, C), (128, 1)))
    maskb = np.ascontiguousarray(
        np.triu(np.ones((128, 128), dtype=np.float32)).astype(ml_dtypes.bfloat16)
    )
    identb = np.ascontiguousarray(np.eye(128, dtype=np.float32).astype(ml_dtypes.bfloat16))
    # selector: rows j<12, sel[j, 64h+m] = (j == h); fp32 bits used as f32r
    sel = np.zeros((128, C), dtype=np.float32)
    for h in range(H):
        sel[h, 64 * h : 64 * (h + 1)] = 1.0

    shared = {
        "wqkv": wqkv16,
        "wproj": wproj16,
        "bqkt": bqkt,
        "bvbc": bvbc,
        "bobc": bobc,
        "maskb": maskb,
        "identb": identb,
        "sel12": sel,
    }
    in_maps = []
    for c in range(NCORES):
        m = dict(shared)
        m["x"] = np.ascontiguousarray(x16[B_LOC * c : B_LOC * (c + 1)])
        in_maps.append(m)
    return in_maps


_PROGRAM = None


def kernel(x, W_qkv, b_qkv, W_proj, b_proj):
    global _PROGRAM
    if _PROGRAM is None:
        _PROGRAM = build_program()
    in_maps = make_in_maps(x, W_qkv, b_qkv, W_proj, b_proj)
    res = run_bass_kernel_spmd(_PROGRAM, in_maps, list(range(NCORES)))
    out = np.concatenate([res.results[c]["out"] for c in range(NCORES)], axis=0)
    return out.astype(np.float32)


if __name__ == "__main__":
    nc = build_program()
    print("built ok; instructions:", sum(len(bb.instructions) for f in nc.m.functions for bb in f.blocks))


# revision 32
# speedup vs baseline: 1.0111x; 1.0111x over previous
"""Causal self-attention (B=16, T=1024, C=768, H=12) on 8 NeuronCores.

Strategy: data-parallel over batch (2 batches per core, no collectives).

v2 redesign for PE density (HAM clock gate wants zero PE-idle gaps):
  - All matmuls in bf16 (fp32 PSUM accumulate): 1 cycle/row at any width,
    FWL weight loads, half SBUF/DMA traffic.
  - Per head: S^T tiles stream through 2 PSUM slots with a 2-tile lookahead
    so the PE never waits on ScalarE's exp.
  - Next pair's QKV projection groups are interleaved INTO the attention
    loop as PE filler (ScalarE exp is the slow engine in phase D; the
    filler keeps the PE warm while exp catches up).
  - Softmax normalization is deferred to a batch-end tail: denominator rows
    are DMA-gathered into l_all[12, T], one reciprocal_approx_fast serves
    the whole batch, and a selector-matrix matmul (eye12 (x) ones64)
    broadcasts each head's reciprocal row to 64 partitions. This kills the
    24 single-lane DVE RECIPROCALs (6.5us each!) of v1.
  - Bias adds moved off ScalarE (DVE tensor_scalar with per-partition AP)
    so ScalarE does exp only.
"""

import os
import numpy as np
from collections import deque
from contextlib import ExitStack

import concourse.bass as bass
import concourse.mybir as mybir
import concourse.tile as tile
from concourse.bass import ds, ts
from concourse.bass_utils import run_bass_kernel_spmd

F32 = mybir.dt.float32
F32R = mybir.dt.float32r
BF = mybir.dt.bfloat16

B, T, C, H = 16, 1024, 768, 12
D = C // H           # 64
NCORES = 8
B_LOC = B // NCORES  # 2
KT = C // 128        # 6 contraction tiles
TT = T // 128        # 8 token tiles
NPAIR = H // 2       # 6 head pairs
EXP = mybir.ActivationFunctionType.Exp


def split_multi_waits(nc):
    """Hoist surplus sync waits onto standalone EventSemaphore instructions.

    The walrus build in this environment rejects any instruction carrying
    more than one sync wait ("Too many sync wait commands"). Engine queues
    execute in order, so waiting on each semaphore in a preceding
    EventSemaphore instruction is equivalent to waiting on all of them at
    the original instruction.
    """
    n_split = 0
    for f in nc.m.functions:
        for blk in f.blocks:
            out = []
            for inst in blk.instructions:
                si = inst.sync_info
                if si is not None and si.on_wait and len(si.on_wait) > 1:
                    waits = list(si.on_wait)
                    for w in waits[:-1]:
                        n_split += 1
                        ev = mybir.InstEventSemaphore(
                            name=f"I-waitsplit-{n_split}",
                            ins=[],
                            outs=[],
                            engine=inst.engine,
                            sync_info=mybir.SyncInfo(on_wait=[w], on_update=[]),
                        )
                        out.append(ev)
                    si.on_wait = waits[-1:]
                out.append(inst)
            blk.instructions[:] = out
    return n_split


def build_program(split_waits=True, level=None):
    """split_waits: apply the multi-wait splitting (required for neuronx-cc
    codegen, but the CoreSim race detector rejects the synthetic
    EventSemaphore instructions — pass False when simulating)."""
    if level is None:
        level = int(os.environ.get("BUILD_LEVEL", "5"))
    nc = bass.Bass()
    x = nc.declare_dram_parameter("x", [B_LOC, T, C], BF, isOutput=False)
    wqkv = nc.declare_dram_parameter("wqkv", [C, 3 * C], BF, isOutput=False)
    wproj = nc.declare_dram_parameter("wproj", [C, C], BF, isOutput=False)
    bqkt = nc.declare_dram_parameter("bqkt", [128, 2 * NPAIR], F32, isOutput=False)
    bvbc = nc.declare_dram_parameter("bvbc", [128, C], F32, isOutput=False)
    bobc = nc.declare_dram_parameter("bobc", [128, C], F32, isOutput=False)
    maskb = nc.declare_dram_parameter("maskb", [128, 128], BF, isOutput=False)
    identb = nc.declare_dram_parameter("identb", [128, 128], BF, isOutput=False)
    sel12 = nc.declare_dram_parameter("sel12", [128, C], F32, isOutput=False)
    out = nc.declare_dram_parameter("out", [B_LOC, T, C], F32, isOutput=True)

    with tile.TileContext(nc) as tc, ExitStack() as ctx, \
            nc.allow_low_precision(reason="bf16 matmul pipeline"):
        consts = ctx.enter_context(tc.tile_pool(name="consts", bufs=1))
        wq_pool = ctx.enter_context(tc.tile_pool(name="wq", bufs=1))
        wp_pool = ctx.enter_context(tc.tile_pool(name="wp", bufs=1))
        xt_pool = ctx.enter_context(tc.tile_pool(name="xt", bufs=1))
        qk_pool = ctx.enter_context(tc.tile_pool(name="qk", bufs=3))
        va_pool = ctx.enter_context(tc.tile_pool(name="va", bufs=1))
        pexp = ctx.enter_context(tc.tile_pool(name="pexp", bufs=4))
        lpool = ctx.enter_context(tc.tile_pool(name="lpool", bufs=1))
        yraw_pool = ctx.enter_context(tc.tile_pool(name="yraw", bufs=1))
        ystg_pool = ctx.enter_context(tc.tile_pool(name="ystg", bufs=2))
        yt_pool = ctx.enter_context(tc.tile_pool(name="yt", bufs=1))
        ostage = ctx.enter_context(tc.tile_pool(name="ostage", bufs=3))
        # PSUM: pco 2x1 bank + st 2x2 banks + ypool 1x2 banks = 8 banks
        pco = ctx.enter_context(tc.tile_pool(name="pco", bufs=2, space="PSUM"))
        st_pool = ctx.enter_context(tc.tile_pool(name="st", bufs=2, space="PSUM"))
        ypool = ctx.enter_context(tc.tile_pool(name="ypool", bufs=1, space="PSUM"))

        # ---- constants -------------------------------------------------
        ident_sb = consts.tile([128, 128], BF)
        nc.sync.dma_start(ident_sb[:], identb[:])
        mask_sb = consts.tile([128, 128], BF)
        nc.sync.dma_start(mask_sb[:], maskb[:])
        xstage = ctx.enter_context(tc.tile_pool(name="xstage", bufs=2))
        bqk_sb = consts.tile([128, 2 * NPAIR], F32)
        bvbc_sb = consts.tile([128, C], F32)
        bobc_sb = consts.tile([128, C], F32)
        sel_sb = consts.tile([128, C], F32R)

        # ---- weights: bf16 from host; one strided DMA per tensor -------
        # (DMA issue on the sync queue costs ~0.65us each, so batch them;
        # emitted AFTER the batch-0 x DMA so phase A is never starved.)
        wqall = wq_pool.tile([128, KT, 3 * C], BF, name="wqall")
        wpall = wp_pool.tile([128, KT, C], BF, name="wpall")
        wq = [wqall[:, k, :] for k in range(KT)]
        wp = [wpall[:, k, :] for k in range(KT)]

        def emit_weight_dmas():
            wqkv3 = wqkv.rearrange("(k p) c -> p k c", p=128)
            nc.sync.dma_start(wqall[:, :, 2 * C :], wqkv3[:, :, 2 * C :])
            nc.sync.dma_start(wqall[:, :, 0 : 2 * C], wqkv3[:, :, 0 : 2 * C])
            nc.sync.dma_start(bqk_sb[:], bqkt[:])
            nc.sync.dma_start(bvbc_sb[:], bvbc[:])
            nc.sync.dma_start(wpall[:], wproj.rearrange("(k p) c -> p k c", p=128))
            nc.sync.dma_start(bobc_sb[:], bobc[:])
            nc.sync.dma_start(sel_sb[:], sel12[:].bitcast(F32R))

        xT = [None] * KT
        va_tiles = [None] * TT
        yraw = {}
        yTt = [None] * NPAIR
        cur_qk = {}
        l_all = None
        l5 = None
        rec = None
        rec5 = None

        # ---- emit helpers ---------------------------------------------
        xfull = [None]

        def emit_Adma(b):
            xs = xstage.tile([128, TT, C], BF, tag="xs", name="xs")
            x3 = x[b].rearrange("(t p) c -> p t c", p=128)
            # first tile separately so phase A starts before the bulk lands
            nc.sync.dma_start(xs[:, 0:1, :], x3[:, 0:1, :])
            nc.sync.dma_start(xs[:, 1:TT, :], x3[:, 1:TT, :])
            xfull[0] = xs

        def emit_Astep(b, tt):
            xs = xfull[0]
            for k in range(KT):
                ptr = pco.tile([128, 512], F32, tag="mm", name="ptr")
                pbf = ptr.bitcast(BF)
                nc.tensor.transpose(pbf[:, 0:128], xs[:, tt, ts(k, 128)], ident_sb[:])
                nc.vector.tensor_copy(xT[k][:, ts(tt, 128)], pbf[:, 0:128])

        def emit_Cgroup(b, tt, half):
            pv = pco.tile([128, 384], F32, tag="mm", name="pv")
            for k in range(KT):
                nc.tensor.matmul(
                    pv[:],
                    lhsT=xT[k][:, ts(tt, 128)],
                    rhs=wq[k][:, ds(2 * C + 384 * half, 384)],
                    start=(k == 0),
                    stop=(k == KT - 1),
                )
            va3 = va_tiles[tt].rearrange("p (h e) -> p h e", e=D + 1)
            nc.vector.tensor_add(
                va3[:, ds(6 * half, 6), 0:D],
                pv[:].rearrange("p (h e) -> p h e", e=D),
                bvbc_sb[:, ds(384 * half, 384)].rearrange("p (h e) -> p h e", e=D),
            )

        def emit_Bgroup(b, p, which, half, xts=None):
            if xts is None:
                xts = xT
            if p not in cur_qk:
                qT = qk_pool.tile([128, T], BF, tag="qT", name="qT")
                kTt = qk_pool.tile([128, T], BF, tag="kT", name="kTt")
                cur_qk[p] = (qT, kTt)
            dst = cur_qk[p][which]
            pq = pco.tile([128, 512], F32, tag="mm", name="pq")
            colbase = 128 * p + which * C
            for k in range(KT):
                nc.tensor.matmul(
                    pq[:],
                    lhsT=wq[k][:, ds(colbase, 128)],
                    rhs=xts[k][:, ds(512 * half, 512)],
                    start=(k == 0),
                    stop=(k == KT - 1),
                )
            j = p + which * NPAIR
            nc.vector.tensor_scalar_add(
                dst[:, ds(512 * half, 512)], pq[:], bqk_sb[:, ds(j, 1)]
            )

        pending = deque()

        def pop_filler(n=1):
            for _ in range(n):
                if pending:
                    pending.popleft()()

        def emit_D_head(b, p, h2):
            h = 2 * p + h2
            pb = D * h2
            qT, kTt = cur_qk[p]
            py = ypool.tile([D + 1, T], F32, tag="y", name="py")
            sts = {}

            def est(i):
                cstart = 128 * i
                wtot = T - cstart
                st = st_pool.tile([128, wtot], F32, tag="st", name="st")
                lc = 0
                while lc < wtot:
                    w = min(512, wtot - lc)
                    nc.tensor.matmul(
                        st[:, ds(lc, w)],
                        lhsT=kTt[ds(pb, D), ts(i, 128)],
                        rhs=qT[ds(pb, D), ds(cstart + lc, w)],
                        start=True,
                        stop=True,
                    )
                    lc += w
                sts[i] = st

            est(0)
            est(1)
            for i in range(TT):
                cstart = 128 * i
                wtot = T - cstart
                pe = pexp.tile([128, wtot], BF, tag="pe", name="pe")
                nc.scalar.activation(pe[:], sts[i][:], EXP, scale=0.125)
                nc.gpsimd.tensor_mul(pe[:, 0:128], pe[:, 0:128], mask_sb[:])
                if i + 2 < TT:
                    est(i + 2)
                if i % 2 == 1:
                    pop_filler()
                cs = cstart
                while cs < T:
                    w = min(512 - (cs % 512), T - cs)
                    nc.tensor.matmul(
                        py[:, ds(cs, w)],
                        lhsT=va_tiles[i][:, ds((D + 1) * h, D + 1)],
                        rhs=pe[:, ds(cs - cstart, w)],
                        start=(i == 0),
                        stop=(i == TT - 1),
                        skip_group_check=True,
                    )
                    cs += w
            # l row: PSUM part-64 -> SBUF part-64 (same-base DVE copy), then
            # SBUF->SBUF DMA does the cross-partition move into l_all[h].
            # Pair 5 lands in its own base-0 tile so its reciprocal can run
            # after the early rows-0:9 reciprocal (start partition must be 0).
            lstg = ystg_pool.tile([D + 1, T], F32, tag="lstg", name="lstg")
            nc.vector.tensor_copy(lstg[ds(D, 1), :], py[ds(D, 1), :])
            if h >= 10:
                nc.sync.dma_start(l5[ds(h - 10, 1), :], lstg[ds(D, 1), :])
            else:
                nc.sync.dma_start(l_all[ds(h, 1), :], lstg[ds(D, 1), :])
            yr = yraw_pool.tile([D, T], BF, tag=f"yr{h}", name=f"yr{h}")
            nc.vector.tensor_copy(yr[:], py[ds(0, D), :])
            yraw[h] = yr

        cur_ystg = {}

        def emit_norm_pair(b, p, half):
            for h2 in range(2):
                h = 2 * p + h2
                if h2 == 0:
                    if half == 0:
                        yt = yt_pool.tile([128, T], BF, tag=f"yT{p}", name=f"yT{p}")
                        yTt[p] = yt
                    tgt = yTt[p]
                else:
                    if half == 0:
                        cur_ystg[p] = ystg_pool.tile(
                            [D, T], BF, tag="ystg", name="ystg", bufs=6
                        )
                    tgt = cur_ystg[p]
                pbt = pco.tile([D, 512], F32, tag="mm", name="pbt")
                if p < NPAIR - 1:
                    sel_ap = sel_sb[ds(0, 10), ds(64 * h, 64)]
                    rec_ap = rec[ds(0, 10), ds(512 * half, 512)]
                else:
                    # sel rows 0-1 over cols 0:128 are eye2 (x) ones64
                    sel_ap = sel_sb[ds(0, 2), ds(64 * h2, 64)]
                    rec_ap = rec5[ds(0, 2), ds(512 * half, 512)]
                nc.tensor.matmul(
                    pbt[:], lhsT=sel_ap, rhs=rec_ap, start=True, stop=True
                )
                nc.vector.tensor_mul(
                    tgt[ds(0, D), ds(512 * half, 512)],
                    yraw[h][:, ds(512 * half, 512)],
                    pbt[:],
                )
                if h2 == 1:
                    nc.sync.dma_start(
                        yTt[p][ds(D, D), ds(512 * half, 512)],
                        tgt[ds(0, D), ds(512 * half, 512)],
                    )

        def emit_Egroup(b, tt, half, yt_fin):
            po = pco.tile([128, 384], F32, tag="mm", name="po")
            for k in range(KT):
                nc.tensor.matmul(
                    po[:],
                    lhsT=yt_fin[k][:, ts(tt, 128)],
                    rhs=wp[k][:, ds(384 * half, 384)],
                    start=(k == 0),
                    stop=(k == KT - 1),
                )
            ot = ostage.tile([128, 384], F32, tag="ot", name="ot")
            nc.vector.tensor_add(ot[:], po[:], bobc_sb[:, ds(384 * half, 384)])
            nc.sync.dma_start(out[b, ts(tt, 128), ds(384 * half, 384)], ot[:])

        def make_CB0(b):
            """Closures: va-tile setup, then C groups with B(pair 0) groups
            interleaved every 4th slot. Resets cur_qk when the first runs."""

            def setup():
                for tt in range(TT):
                    va = va_pool.tile(
                        [128, H * (D + 1)], BF, tag=f"va{tt}", name=f"va{tt}"
                    )
                    va3 = va.rearrange("p (h e) -> p h e", e=D + 1)
                    nc.vector.memset(va3[:, :, D : D + 1], 1.0)
                    va_tiles[tt] = va

            items = [setup]
            for tt in range(TT):
                for half in range(2):
                    items.append(lambda tt=tt, half=half: emit_Cgroup(b, tt, half))
            return items

        def emit_CB0(b):
            for f in make_CB0(b):
                f()

        # ---- program ---------------------------------------------------
        def emit_batch(b, skipA, skipCB=False):
            nonlocal l_all, l5, rec, rec5
            if not skipA:
                for k in range(KT):
                    xT[k] = xt_pool.tile([128, T], BF, tag=f"xT{k}", name=f"xT{k}")
                emit_Adma(b)
                if b == 0:
                    emit_weight_dmas()
                for tt in range(TT):
                    emit_Astep(b, tt)
            if level <= 1:
                for k in range(KT):
                    nc.sync.dma_start(out[b, ts(k, 128), 0:512], xT[k][:, :].bitcast(F32))
                return
            if not skipCB:
                cur_qk.clear()
                cb_items = make_CB0(b)
                for n, f in enumerate(cb_items):
                    f()
                    if n in (4, 8, 12, 16):
                        g = (n - 4) // 4
                        emit_Bgroup(b, 0, g // 2, g % 2)
            if level <= 2:
                for tt in range(TT):
                    nc.sync.dma_start(out[b, ts(tt, 128), 0:390], va_tiles[tt][:, :].bitcast(F32))
                return
            if level <= 3:
                for p in range(1, NPAIR):
                    for g in range(4):
                        emit_Bgroup(b, p, g // 2, g % 2)
                for p in range(NPAIR):
                    qT, kTt = cur_qk[p]
                    nc.sync.dma_start(out[b, ts(p, 128), 0:512], qT[:, :].bitcast(F32))
                    nc.sync.dma_start(out[b, ts(p, 128), 512:768], kTt[:, 0:512].bitcast(F32))
                return
            l_all = lpool.tile([10, T], F32, tag="l_all", name="l_all")
            l5 = lpool.tile([2, T], F32, tag="l5", name="l5")
            for p in range(NPAIR):
                if p < NPAIR - 1:
                    pnext = p + 1
                    xsnap = list(xT)
                    for g in range(4):
                        pending.append(
                            lambda g=g, pnext=pnext, xs=xsnap: emit_Bgroup(
                                b, pnext, g // 2, g % 2, xs
                            )
                        )
                    if p == NPAIR - 2 and b == 0 and level >= 5:
                        # stage half the next batch's transposes here so the
                        # pair-5 DVE queue stays short for the l/recip chain
                        for k in range(KT):
                            xT[k] = xt_pool.tile(
                                [128, T], BF, tag=f"xT{k}", name=f"xT{k}"
                            )
                        emit_Adma(1)
                        for tt in range(4):
                            pending.append(lambda tt=tt: emit_Astep(1, tt))
                elif b == 0 and level >= 5:
                    for tt in range(4, TT):
                        pending.append(lambda tt=tt: emit_Astep(1, tt))
                    cur_qk.pop(0, None)
                    xsnap = list(xT)
                    for g in range(4):
                        pending.append(
                            lambda g=g, xs=xsnap: emit_Bgroup(
                                1, 0, g // 2, g % 2, xs
                            )
                        )
                if p == NPAIR - 1:
                    # pairs 0-4 denominators are in: reciprocal them during
                    # the last pair's attention so only pair 5's rows remain
                    rec = lpool.tile([10, T], F32R, tag="rec", name="rec")
                    rec5 = lpool.tile([2, T], F32R, tag="rec5", name="rec5")
                    nc.vector.reciprocal(rec[ds(0, 10), 0:512], l_all[ds(0, 10), 0:512])
                    nc.vector.reciprocal(rec[ds(0, 10), 512:T], l_all[ds(0, 10), 512:T])
                emit_D_head(b, p, 0)
                emit_D_head(b, p, 1)
            pop_filler(len(pending))
            if b == 0:
                keep = cur_qk.get(0)
                cur_qk.clear()
                if keep is not None:
                    cur_qk[0] = keep
            nc.vector.reciprocal(rec5[ds(0, 2), 0:512], l5[ds(0, 2), 0:512])
            nc.vector.reciprocal(rec5[ds(0, 2), 512:T], l5[ds(0, 2), 512:T])
            for p in range(NPAIR):
                emit_norm_pair(b, p, 0)
            if b == 0:
                emit_CB0(1)
            if level <= 4:
                for p in range(NPAIR):
                    emit_norm_pair(b, p, 1)
                for p in range(NPAIR):
                    nc.sync.dma_start(out[b, ts(p, 128), 0:512], yTt[p][:, :].bitcast(F32))
                return
            yt_fin = list(yTt)
            # E for tt 0-3 only needs the first 512 cols of yTt: interleave
            # with the second normalization half so the PE never waits.
            for tt in range(4):
                for half in range(2):
                    emit_Egroup(b, tt, half, yt_fin)
            for p in range(NPAIR):
                emit_norm_pair(b, p, 1)
            for tt in range(4, TT):
                for half in range(2):
                    emit_Egroup(b, tt, half, yt_fin)

        emit_batch(0, skipA=False)
        if level >= 5:
            emit_batch(1, skipA=True, skipCB=True)

    if split_waits:
        split_multi_waits(nc)
    return nc


def make_in_maps(x, W_qkv, b_qkv, W_proj, b_proj):
    x = np.ascontiguousarray(np.asarray(x, dtype=np.float32))
    W_qkv = np.ascontiguousarray(np.asarray(W_qkv, dtype=np.float32))
    b_qkv = np.asarray(b_qkv, dtype=np.float32)
    W_proj = np.ascontiguousarray(np.asarray(W_proj, dtype=np.float32))
    b_proj = np.asarray(b_proj, dtype=np.float32)

    import ml_dtypes

    x16 = x.astype(ml_dtypes.bfloat16)
    wqkv16 = np.ascontiguousarray(W_qkv.astype(ml_dtypes.bfloat16))
    wproj16 = np.ascontiguousarray(W_proj.astype(ml_dtypes.bfloat16))
    bqkt = np.ascontiguousarray(b_qkv[: 2 * C].reshape(2 * NPAIR, 128).T)
    bvbc = np.ascontiguousarray(np.tile(b_qkv[2 * C :].reshape(1, C), (128, 1)))
    bobc = np.ascontiguousarray(np.tile(b_proj.reshape(1, C), (128, 1)))
    maskb = np.ascontiguousarray(
        np.triu(np.ones((128, 128), dtype=np.float32)).astype(ml_dtypes.bfloat16)
    )
    identb = np.ascontiguousarray(np.eye(128, dtype=np.float32).astype(ml_dtypes.bfloat16))
    # selector: rows j<12, sel[j, 64h+m] = (j == h); fp32 bits used as f32r
    sel = np.zeros((128, C), dtype=np.float32)
    for h in range(H):
        sel[h, 64 * h : 64 * (h + 1)] = 1.0

    shared = {
        "wqkv": wqkv16,
        "wproj": wproj16,
        "bqkt": bqkt,
        "bvbc": bvbc,
        "bobc": bobc,
        "maskb": maskb,
        "identb": identb,
        "sel12": sel,
    }
    in_maps = []
    for c in range(NCORES):
        m = dict(shared)
        m["x"] = np.ascontiguousarray(x16[B_LOC * c : B_LOC * (c + 1)])
        in_maps.append(m)
    return in_maps


_PROGRAM = None


def kernel(x, W_qkv, b_qkv, W_proj, b_proj):
    global _PROGRAM
    if _PROGRAM is None:
        _PROGRAM = build_program()
    in_maps = make_in_maps(x, W_qkv, b_qkv, W_proj, b_proj)
    res = run_bass_kernel_spmd(_PROGRAM, in_maps, list(range(NCORES)))
    out = np.concatenate([res.results[c]["out"] for c in range(NCORES)], axis=0)
    return out.astype(np.float32)


if __name__ == "__main__":
    nc = build_program()
    print("built ok; instructions:", sum(len(bb.instructions) for f in nc.m.functions for bb in f.blocks))
